# revision 3
# baseline (speedup 1.0000x reference)
"""Maxwell viscoelastic recurrence (explicit Euler) on 8 TRN2 NeuronCores.

Math: with E_inf=0.5, E=2.0, eta=1.0,
    d_n        = eps_n - gamma_n
    sig_n      = 0.5*eps_n + 2*d_n              = 2.5*eps_n - 2*gamma_n
    gamma_{n+1}= gamma_n + 2*dt_n*d_n           = (1-2*dt_n)*gamma_n + 2*dt_n*eps_n

Substituting h_n = -2*gamma_n gives a first-order linear scan:
    h_{n+1} = a_n*h_n + b_n,  a_n = 1-2*dt_n,  b_n = -4*dt_n*eps_n,  h_0 = 0
    sig_n   = 2.5*eps_n + h_n

which maps directly onto the VectorEngine's tensor_tensor_scan
(state = data0*state + data1 along the free axis, one recurrence per
partition lane).

Sharding: pure data parallel over the batch axis (2048 rows -> 256 per
core). Per core the 256 rows form two 128-partition tiles; T=8192 is
streamed in column chunks with the scan carry chained via the chunk's
leading h column.
"""

import numpy as np

B, T = 2048, 8192
N_CORES = 8
B_LOCAL = B // N_CORES  # 256
P = 128                 # SBUF partitions
C = 2048                # T-chunk columns
N_PT = B_LOCAL // P     # partition tiles per core
N_CH = T // C           # chunks along T

_cache = {}


def _build():
    import concourse.tile as tile
    from concourse import bacc, mybir

    f32 = mybir.dt.float32
    mult = mybir.AluOpType.mult
    add = mybir.AluOpType.add

    nc = bacc.Bacc("TRN2", target_bir_lowering=False, debug=False,
                   num_devices=N_CORES)
    eps_d = nc.dram_tensor("eps", [B_LOCAL, T], f32, kind="ExternalInput").ap()
    dts_d = nc.dram_tensor("dts", [B_LOCAL, T], f32, kind="ExternalInput").ap()
    out_d = nc.dram_tensor("out", [B_LOCAL, T], f32, kind="ExternalOutput").ap()

    # Engine budget per [128, C=2048] f32 chunk (8 chunks/core):
    #   DVE scan: ~4.5us (2 cyc/elem, HW bubble), DVE stt: ~2.3us,
    #   Pool TT: ~4.5us, ACT activation: ~1.3us, DMA: ~8.8us/chunk.
    # Balance the 16 two-input ops (8 b, 8 sig) so no compute engine
    # exceeds DMA: all 8 b on Pool, 3 sig on Pool, 5 sig on DVE.
    SIG_ON_POOL = {1, 4, 6}

    with tile.TileContext(nc) as tc:
        with (
            tc.tile_pool(name="io", bufs=3) as io_pool,
            tc.tile_pool(name="ab", bufs=3) as ab_pool,
            tc.tile_pool(name="h", bufs=2 * N_PT) as h_pool,
            tc.tile_pool(name="misc", bufs=1) as misc_pool,
        ):
            one = misc_pool.tile([P, 1], f32, tag="one")
            nc.gpsimd.memset(one[:], 1.0)

            h_prev = [None] * N_PT
            for c in range(N_CH):
                for pt in range(N_PT):
                    i = c * N_PT + pt
                    r0 = pt * P
                    rows = slice(r0, r0 + P)
                    cols = slice(c * C, (c + 1) * C)

                    eps_t = io_pool.tile([P, C], f32, tag="eps")
                    nc.sync.dma_start(eps_t[:], eps_d[rows, cols])
                    dts_t = io_pool.tile([P, C], f32, tag="dts")
                    nc.sync.dma_start(dts_t[:], dts_d[rows, cols])

                    # a = 1 - 2*dt   (ScalarE: Identity(dt*-2 + 1))
                    a_t = ab_pool.tile([P, C], f32, tag="a")
                    nc.scalar.activation(
                        a_t[:], dts_t[:],
                        mybir.ActivationFunctionType.Identity,
                        bias=one[:], scale=-2.0,
                    )
                    # b = -4*dt*eps  (ACT scales, Pool multiplies)
                    m4dt_t = ab_pool.tile([P, C], f32, tag="m4dt")
                    nc.scalar.mul(m4dt_t[:], dts_t[:], -4.0)
                    b_t = ab_pool.tile([P, C], f32, tag="b")
                    nc.gpsimd.tensor_tensor(b_t[:], m4dt_t[:], eps_t[:], mult)

                    # h chunk: col 0 carries h at chunk start, scan fills 1..C
                    h_t = h_pool.tile([P, C + 1], f32, tag="h")
                    if c == 0:
                        nc.gpsimd.memset(h_t[:, 0:1], 0.0)
                    else:
                        nc.scalar.copy(h_t[:, 0:1], h_prev[pt][:, C:C + 1])
                    nc.vector.tensor_tensor_scan(
                        h_t[:, 1:C + 1], a_t[:], b_t[:], h_t[:, 0:1],
                        mult, add)
                    h_prev[pt] = h_t

                    # sig = 2.5*eps + h
                    sig_t = io_pool.tile([P, C], f32, tag="sig")
                    if i in SIG_ON_POOL:
                        eps25_t = ab_pool.tile([P, C], f32, tag="eps25")
                        nc.scalar.mul(eps25_t[:], eps_t[:], 2.5)
                        nc.gpsimd.tensor_tensor(
                            sig_t[:], eps25_t[:], h_t[:, 0:C], add)
                    else:
                        nc.vector.scalar_tensor_tensor(
                            sig_t[:], eps_t[:], 2.5, h_t[:, 0:C], mult, add)
                    nc.sync.dma_start(out_d[rows, cols], sig_t[:])

    nc.compile()
    return nc


def kernel(eps: np.ndarray, dts: np.ndarray) -> np.ndarray:
    from concourse.bass_utils import run_bass_kernel_spmd

    e = np.ascontiguousarray(eps.reshape(B, T), dtype=np.float32)
    d = np.ascontiguousarray(dts.reshape(B, T), dtype=np.float32)

    if "nc" not in _cache:
        _cache["nc"] = _build()
    nc = _cache["nc"]

    in_maps = [
        {"eps": e[i * B_LOCAL:(i + 1) * B_LOCAL],
         "dts": d[i * B_LOCAL:(i + 1) * B_LOCAL]}
        for i in range(N_CORES)
    ]
    res = run_bass_kernel_spmd(nc, in_maps, core_ids=list(range(N_CORES)))
    out = np.concatenate(
        [np.asarray(res.results[i]["out"]) for i in range(N_CORES)], axis=0)
    return out.reshape(B, T, 1)


# revision 4
# speedup vs baseline: 1.1349x; 1.1349x over previous
"""Maxwell viscoelastic recurrence (explicit Euler) on 8 TRN2 NeuronCores.

Math: with E_inf=0.5, E=2.0, eta=1.0,
    d_n        = eps_n - gamma_n
    sig_n      = 0.5*eps_n + 2*d_n              = 2.5*eps_n - 2*gamma_n
    gamma_{n+1}= gamma_n + 2*dt_n*d_n           = (1-2*dt_n)*gamma_n + 2*dt_n*eps_n

Substituting h_n = -2*gamma_n gives a first-order linear scan:
    h_{n+1} = a_n*h_n + b_n,  a_n = 1-2*dt_n,  b_n = -4*dt_n*eps_n,  h_0 = 0
    sig_n   = 2.5*eps_n + h_n

which maps directly onto the VectorEngine's tensor_tensor_scan
(state = data0*state + data1 along the free axis, one recurrence per
partition lane).

Sharding: pure data parallel over the batch axis (2048 rows -> 256 per
core). Per core the 256 rows form two 128-partition tiles; T=8192 is
streamed in column chunks with the scan carry chained via the chunk's
leading h column.
"""

import numpy as np

B, T = 2048, 8192
N_CORES = 8
B_LOCAL = B // N_CORES  # 256
P = 128                 # SBUF partitions
C = 2048                # T-chunk columns
N_PT = B_LOCAL // P     # partition tiles per core
N_CH = T // C           # chunks along T

_cache = {}


def _build():
    import concourse.tile as tile
    from concourse import bacc, mybir

    f32 = mybir.dt.float32
    mult = mybir.AluOpType.mult
    add = mybir.AluOpType.add

    nc = bacc.Bacc("TRN2", target_bir_lowering=False, debug=False,
                   num_devices=N_CORES)
    eps_d = nc.dram_tensor("eps", [B_LOCAL, T], f32, kind="ExternalInput").ap()
    dts_d = nc.dram_tensor("dts", [B_LOCAL, T], f32, kind="ExternalInput").ap()
    out_d = nc.dram_tensor("out", [B_LOCAL, T], f32, kind="ExternalOutput").ap()

    # Engine budget per [128, C=2048] f32 chunk (8 chunks/core):
    #   DVE scan: ~4.5us (2 cyc/elem, HW feedback bubble), DVE stt: ~2.3us,
    #   Pool TT: ~5us, ACT activation: ~1.3us, DMA: ~8.8us/chunk.
    # The scan chain (dts -> b -> scan -> next scan) must stay on fast
    # engines: a on ACT, b on DVE. sig is off-chain, so it can absorb
    # Pool's latency — run it there except for the last chunk (tail).
    N_IT = N_CH * N_PT

    with tile.TileContext(nc) as tc:
        with (
            tc.tile_pool(name="io", bufs=3) as io_pool,
            tc.tile_pool(name="ab", bufs=3) as ab_pool,
            tc.tile_pool(name="h", bufs=2 * N_PT) as h_pool,
            tc.tile_pool(name="misc", bufs=1) as misc_pool,
        ):
            one = misc_pool.tile([P, 1], f32, tag="one")
            nc.gpsimd.memset(one[:], 1.0)
            zero = misc_pool.tile([P, 1], f32, tag="zero")
            nc.gpsimd.memset(zero[:], 0.0)

            h_prev = [None] * N_PT
            for c in range(N_CH):
                for pt in range(N_PT):
                    i = c * N_PT + pt
                    r0 = pt * P
                    rows = slice(r0, r0 + P)
                    cols = slice(c * C, (c + 1) * C)

                    eps_t = io_pool.tile([P, C], f32, tag="eps")
                    nc.sync.dma_start(eps_t[:], eps_d[rows, cols])
                    dts_t = io_pool.tile([P, C], f32, tag="dts")
                    nc.sync.dma_start(dts_t[:], dts_d[rows, cols])

                    # a = 1 - 2*dt   (ScalarE: Identity(dt*-2 + 1))
                    a_t = ab_pool.tile([P, C], f32, tag="a")
                    nc.scalar.activation(
                        a_t[:], dts_t[:],
                        mybir.ActivationFunctionType.Identity,
                        bias=one[:], scale=-2.0,
                    )
                    # b = -4*dt*eps  (VectorE, feeds the scan)
                    b_t = ab_pool.tile([P, C], f32, tag="b")
                    nc.vector.scalar_tensor_tensor(
                        b_t[:], dts_t[:], -4.0, eps_t[:], mult, mult)

                    # h chunk: col 0 carries h at chunk start, scan fills 1..C
                    h_t = h_pool.tile([P, C + 1], f32, tag="h")
                    if c == 0:
                        nc.gpsimd.memset(h_t[:, 0:1], 0.0)
                    else:
                        nc.scalar.activation(
                            h_t[:, 0:1], h_prev[pt][:, C:C + 1],
                            mybir.ActivationFunctionType.Identity,
                            bias=zero[:], scale=1.0)
                    nc.vector.tensor_tensor_scan(
                        h_t[:, 1:C + 1], a_t[:], b_t[:], h_t[:, 0:1],
                        mult, add)
                    h_prev[pt] = h_t

                    # sig = 2.5*eps + h
                    sig_t = io_pool.tile([P, C], f32, tag="sig")
                    if i < N_IT - 1:
                        eps25_t = ab_pool.tile([P, C], f32, tag="eps25")
                        nc.scalar.activation(
                            eps25_t[:], eps_t[:],
                            mybir.ActivationFunctionType.Identity,
                            bias=zero[:], scale=2.5)
                        nc.gpsimd.tensor_tensor(
                            sig_t[:], eps25_t[:], h_t[:, 0:C], add)
                    else:
                        nc.vector.scalar_tensor_tensor(
                            sig_t[:], eps_t[:], 2.5, h_t[:, 0:C], mult, add)
                    nc.sync.dma_start(out_d[rows, cols], sig_t[:])

    nc.compile()
    return nc


def kernel(eps: np.ndarray, dts: np.ndarray) -> np.ndarray:
    from concourse.bass_utils import run_bass_kernel_spmd

    e = np.ascontiguousarray(eps.reshape(B, T), dtype=np.float32)
    d = np.ascontiguousarray(dts.reshape(B, T), dtype=np.float32)

    if "nc" not in _cache:
        _cache["nc"] = _build()
    nc = _cache["nc"]

    in_maps = [
        {"eps": e[i * B_LOCAL:(i + 1) * B_LOCAL],
         "dts": d[i * B_LOCAL:(i + 1) * B_LOCAL]}
        for i in range(N_CORES)
    ]
    res = run_bass_kernel_spmd(nc, in_maps, core_ids=list(range(N_CORES)))
    out = np.concatenate(
        [np.asarray(res.results[i]["out"]) for i in range(N_CORES)], axis=0)
    return out.reshape(B, T, 1)


# revision 8
# speedup vs baseline: 1.2196x; 1.0746x over previous
"""Maxwell viscoelastic recurrence (explicit Euler) on 8 TRN2 NeuronCores.

Math: with E_inf=0.5, E=2.0, eta=1.0,
    d_n        = eps_n - gamma_n
    sig_n      = 0.5*eps_n + 2*d_n              = 2.5*eps_n - 2*gamma_n
    gamma_{n+1}= gamma_n + 2*dt_n*d_n           = (1-2*dt_n)*gamma_n + 2*dt_n*eps_n

Substituting h_n = -2*gamma_n gives a first-order linear scan:
    h_{n+1} = a_n*h_n + b_n,  a_n = 1-2*dt_n,  b_n = -4*dt_n*eps_n,  h_0 = 0
    sig_n   = 2.5*eps_n + h_n

which maps onto the VectorEngine's tensor_tensor_scan (state =
data0*state + data1 along the free axis, one recurrence per partition
lane; ~2 cycles/element due to the HW feedback bubble).

Sharding: pure data parallel over the batch axis (2048 rows -> 256 per
core). Per core the 256 rows form two 128-partition tiles; T=8192 is
streamed in 2048-column chunks with the scan carry chained through the
chunk's leading h column.

Engine assignment per chunk (DMA ~8.8us/chunk is the roofline):
    ACT : a = 1-2*dt (~1.3us) + carry copy
    DVE : b = -4*dt*eps (stt ~2.3us) + scan (~4.5us)
    PE  : sig = 2.5*eps + h as two accumulating matmuls per 512-col
          PSUM bank with constant weights [2.5*I | I] (PE is otherwise
          idle; Pool shares SBUF ports with DVE so it is NOT used)
    DMA out streams sig straight from PSUM.
"""

import numpy as np

B, T = 2048, 8192
N_CORES = 8
B_LOCAL = B // N_CORES  # 256
P = 128                 # SBUF partitions
C = 2048                # T-chunk columns
MM_N = 512              # matmul free-dim (one PSUM bank)
N_PT = B_LOCAL // P     # partition tiles per core
N_CH = T // C           # chunks along T

_cache = {}


def _build():
    import concourse.tile as tile
    from concourse import bacc, mybir

    f32 = mybir.dt.float32
    mult = mybir.AluOpType.mult
    add = mybir.AluOpType.add

    nc = bacc.Bacc("TRN2", target_bir_lowering=False, debug=False,
                   num_devices=N_CORES)
    eps_d = nc.dram_tensor("eps", [B_LOCAL, T], f32, kind="ExternalInput").ap()
    dts_d = nc.dram_tensor("dts", [B_LOCAL, T], f32, kind="ExternalInput").ap()
    wid_d = nc.dram_tensor("wid", [P, 2 * P], f32, kind="ExternalInput").ap()
    out_d = nc.dram_tensor("out", [B_LOCAL, T], f32, kind="ExternalOutput").ap()

    with tile.TileContext(nc) as tc:
        with (
            tc.tile_pool(name="io", bufs=3) as io_pool,
            tc.tile_pool(name="ab", bufs=3) as ab_pool,
            tc.tile_pool(name="h", bufs=2 * N_PT) as h_pool,
            tc.tile_pool(name="psum", bufs=2, space="PSUM") as psum_pool,
            tc.tile_pool(name="misc", bufs=1) as misc_pool,
        ):
            one = misc_pool.tile([P, 1], f32, tag="one")
            nc.gpsimd.memset(one[:], 1.0)
            zero = misc_pool.tile([P, 1], f32, tag="zero")
            nc.gpsimd.memset(zero[:], 0.0)
            wid_t = misc_pool.tile([P, 2 * P], f32, tag="wid")
            nc.sync.dma_start(wid_t[:], wid_d[:])

            h_prev = [None] * N_PT
            for c in range(N_CH):
                for pt in range(N_PT):
                    r0 = pt * P
                    rows = slice(r0, r0 + P)
                    cols = slice(c * C, (c + 1) * C)

                    eps_t = io_pool.tile([P, C], f32, tag="eps")
                    nc.sync.dma_start(eps_t[:], eps_d[rows, cols])
                    dts_t = io_pool.tile([P, C], f32, tag="dts")
                    nc.sync.dma_start(dts_t[:], dts_d[rows, cols])

                    # a = 1 - 2*dt   (ScalarE: Identity(dt*-2 + 1))
                    a_t = ab_pool.tile([P, C], f32, tag="a")
                    nc.scalar.activation(
                        a_t[:], dts_t[:],
                        mybir.ActivationFunctionType.Identity,
                        bias=one[:], scale=-2.0,
                    )
                    # b = -4*dt*eps  (VectorE, feeds the scan)
                    b_t = ab_pool.tile([P, C], f32, tag="b")
                    nc.vector.scalar_tensor_tensor(
                        b_t[:], dts_t[:], -4.0, eps_t[:], mult, mult)

                    # h chunk: col 0 carries h at chunk start, scan fills 1..C
                    h_t = h_pool.tile([P, C + 1], f32, tag="h")
                    if c == 0:
                        nc.gpsimd.memset(h_t[:, 0:1], 0.0)
                    else:
                        nc.scalar.activation(
                            h_t[:, 0:1], h_prev[pt][:, C:C + 1],
                            mybir.ActivationFunctionType.Identity,
                            bias=zero[:], scale=1.0)
                    nc.vector.tensor_tensor_scan(
                        h_t[:, 1:C + 1], a_t[:], b_t[:], h_t[:, 0:1],
                        mult, add)
                    h_prev[pt] = h_t

                    # sig = 2.5*eps + h on the TensorEngine:
                    # psum = (2.5*I)^T @ eps, then += I^T @ h, per bank.
                    sig_p = psum_pool.tile([P, C], f32, tag="sig")
                    for g in range(C // MM_N):
                        gs = slice(g * MM_N, (g + 1) * MM_N)
                        nc.tensor.matmul(
                            sig_p[:, gs], wid_t[:, 0:P], eps_t[:, gs],
                            start=True, stop=False)
                        nc.tensor.matmul(
                            sig_p[:, gs], wid_t[:, P:2 * P], h_t[:, gs],
                            start=False, stop=True)
                    # PSUM is not DMA-readable: bounce through SBUF via ACT
                    sig_t = io_pool.tile([P, C], f32, tag="sig")
                    nc.scalar.activation(
                        sig_t[:], sig_p[:],
                        mybir.ActivationFunctionType.Identity,
                        bias=zero[:], scale=1.0)
                    nc.sync.dma_start(out_d[rows, cols], sig_t[:])

    nc.compile()
    return nc


def make_in_maps(e, d, wid=None):
    if wid is None:
        wid = np.zeros((P, 2 * P), dtype=np.float32)
        wid[:, :P] = 2.5 * np.eye(P, dtype=np.float32)
        wid[:, P:] = np.eye(P, dtype=np.float32)
    return [
        {"eps": e[i * B_LOCAL:(i + 1) * B_LOCAL],
         "dts": d[i * B_LOCAL:(i + 1) * B_LOCAL],
         "wid": wid}
        for i in range(N_CORES)
    ]


def kernel(eps: np.ndarray, dts: np.ndarray) -> np.ndarray:
    from concourse.bass_utils import run_bass_kernel_spmd

    e = np.ascontiguousarray(eps.reshape(B, T), dtype=np.float32)
    d = np.ascontiguousarray(dts.reshape(B, T), dtype=np.float32)
    wid = np.zeros((P, 2 * P), dtype=np.float32)
    wid[:, :P] = 2.5 * np.eye(P, dtype=np.float32)
    wid[:, P:] = np.eye(P, dtype=np.float32)

    if "nc" not in _cache:
        _cache["nc"] = _build()
    nc = _cache["nc"]

    in_maps = make_in_maps(e, d, wid)
    res = run_bass_kernel_spmd(nc, in_maps, core_ids=list(range(N_CORES)))
    out = np.concatenate(
        [np.asarray(res.results[i]["out"]) for i in range(N_CORES)], axis=0)
    return out.reshape(B, T, 1)


# revision 9
# speedup vs baseline: 1.3198x; 1.0822x over previous
"""Maxwell viscoelastic recurrence (explicit Euler) on 8 TRN2 NeuronCores.

Math: with E_inf=0.5, E=2.0, eta=1.0,
    d_n        = eps_n - gamma_n
    sig_n      = 0.5*eps_n + 2*d_n              = 2.5*eps_n - 2*gamma_n
    gamma_{n+1}= gamma_n + 2*dt_n*d_n           = (1-2*dt_n)*gamma_n + 2*dt_n*eps_n

sig itself satisfies a first-order linear recurrence (substitute gamma
in terms of sig):
    sig_{n+1} = a_n*sig_n + q_n
    a_n = 1 - 2*dt_n
    q_n = 2.5*eps_{n+1} - (2.5 - dt_n)*eps_n
    sig_0 = 2.5*eps_0

which maps directly onto the VectorEngine's tensor_tensor_scan (state =
data0*state + data1 along the free axis, one recurrence per partition
lane, ~2 cycles/element). The scan output IS the kernel output, so it
streams straight to the store DMA, and the cross-chunk carry is just the
previous chunk's last scan column used as the next scan's initial value
(no copies).

Sharding: pure data parallel over the batch axis (2048 rows -> 256 per
core). Per core the 256 rows form two 128-partition tiles; T=8192 is
streamed in 2048-column chunks.

Engine assignment per chunk (DMA ~8.8us/chunk is the roofline):
    ACT : a = 1-2*dt (~1.3us)
    DVE : r = (dt-2.5)*eps, q = 2.5*eps_{+1} + r  (stt, ~2.3us each)
          + scan (~4.5us)  => ~9us/chunk, just above DMA
    Pool/PE: idle (Pool shares SBUF ports with DVE and slows it; PE
          fp32 matmul is ~1.7 cyc/col + weight loads - both measured
          slower than keeping the work on DVE).
"""

import numpy as np

B, T = 2048, 8192
N_CORES = 8
B_LOCAL = B // N_CORES  # 256
P = 128                 # SBUF partitions
C = 2048                # T-chunk columns
N_PT = B_LOCAL // P     # partition tiles per core
N_CH = T // C           # chunks along T

_cache = {}


def _build():
    import concourse.tile as tile
    from concourse import bacc, mybir

    f32 = mybir.dt.float32
    mult = mybir.AluOpType.mult
    add = mybir.AluOpType.add
    sub = mybir.AluOpType.subtract

    nc = bacc.Bacc("TRN2", target_bir_lowering=False, debug=False,
                   num_devices=N_CORES)
    eps_d = nc.dram_tensor("eps", [B_LOCAL, T], f32, kind="ExternalInput").ap()
    dts_d = nc.dram_tensor("dts", [B_LOCAL, T], f32, kind="ExternalInput").ap()
    out_d = nc.dram_tensor("out", [B_LOCAL, T], f32, kind="ExternalOutput").ap()

    with tile.TileContext(nc) as tc:
        with (
            tc.tile_pool(name="io", bufs=4) as io_pool,
            tc.tile_pool(name="aux", bufs=3) as aux_pool,
            tc.tile_pool(name="sig", bufs=2 * N_PT) as sig_pool,
            tc.tile_pool(name="misc", bufs=1) as misc_pool,
        ):
            one = misc_pool.tile([P, 1], f32, tag="one")
            nc.gpsimd.memset(one[:], 1.0)
            zero = misc_pool.tile([P, 1], f32, tag="zero")
            nc.gpsimd.memset(zero[:], 0.0)

            sig_prev = [None] * N_PT
            for c in range(N_CH):
                for pt in range(N_PT):
                    r0 = pt * P
                    rows = slice(r0, r0 + P)
                    last = c == N_CH - 1

                    # eps with one column of lookahead for q
                    eps_t = io_pool.tile([P, C + 1], f32, tag="eps")
                    if last:
                        nc.sync.dma_start(
                            eps_t[:, 0:C], eps_d[rows, c * C:(c + 1) * C])
                        nc.gpsimd.memset(eps_t[:, C:C + 1], 0.0)
                    else:
                        nc.sync.dma_start(
                            eps_t[:], eps_d[rows, c * C:c * C + C + 1])
                    dts_t = io_pool.tile([P, C], f32, tag="dts")
                    nc.sync.dma_start(
                        dts_t[:], dts_d[rows, c * C:(c + 1) * C])

                    # a = 1 - 2*dt   (ScalarE: Identity(dt*-2 + 1))
                    a_t = aux_pool.tile([P, C], f32, tag="a")
                    nc.scalar.activation(
                        a_t[:], dts_t[:],
                        mybir.ActivationFunctionType.Identity,
                        bias=one[:], scale=-2.0,
                    )
                    # r = (dt - 2.5)*eps ; q = 2.5*eps_{+1} + r  (VectorE)
                    r_t = aux_pool.tile([P, C], f32, tag="r")
                    nc.vector.scalar_tensor_tensor(
                        r_t[:], dts_t[:], 2.5, eps_t[:, 0:C], sub, mult)
                    q_t = aux_pool.tile([P, C], f32, tag="q")
                    nc.vector.scalar_tensor_tensor(
                        q_t[:], eps_t[:, 1:C + 1], 2.5, r_t[:], mult, add)

                    # scan: sig_t[:, j] = sig_{c*C + j}; col 0 only used for
                    # the chunk-0 seed, later chunks chain straight off the
                    # previous tile's last column.
                    sig_t = sig_pool.tile([P, C + 1], f32, tag="sig")
                    if c == 0:
                        nc.scalar.activation(
                            sig_t[:, 0:1], eps_t[:, 0:1],
                            mybir.ActivationFunctionType.Identity,
                            bias=zero[:], scale=2.5)
                        initial = sig_t[:, 0:1]
                    else:
                        initial = sig_prev[pt][:, C:C + 1]
                    nc.vector.tensor_tensor_scan(
                        sig_t[:, 1:C + 1], a_t[:], q_t[:], initial,
                        mult, add)
                    sig_prev[pt] = sig_t

                    # store: chunk 0 covers cols 0..C, later chunks 1..C,
                    # the last chunk 1..C-1 (sig_T is never emitted)
                    if c == 0:
                        nc.sync.dma_start(
                            out_d[rows, 0:C + 1], sig_t[:, 0:C + 1])
                    elif last:
                        nc.sync.dma_start(
                            out_d[rows, c * C + 1:T], sig_t[:, 1:C])
                    else:
                        nc.sync.dma_start(
                            out_d[rows, c * C + 1:(c + 1) * C + 1],
                            sig_t[:, 1:C + 1])

    nc.compile()
    return nc


def make_in_maps(e, d):
    return [
        {"eps": e[i * B_LOCAL:(i + 1) * B_LOCAL],
         "dts": d[i * B_LOCAL:(i + 1) * B_LOCAL]}
        for i in range(N_CORES)
    ]


def kernel(eps: np.ndarray, dts: np.ndarray) -> np.ndarray:
    from concourse.bass_utils import run_bass_kernel_spmd

    e = np.ascontiguousarray(eps.reshape(B, T), dtype=np.float32)
    d = np.ascontiguousarray(dts.reshape(B, T), dtype=np.float32)

    if "nc" not in _cache:
        _cache["nc"] = _build()
    nc = _cache["nc"]

    in_maps = make_in_maps(e, d)
    res = run_bass_kernel_spmd(nc, in_maps, core_ids=list(range(N_CORES)))
    out = np.concatenate(
        [np.asarray(res.results[i]["out"]) for i in range(N_CORES)], axis=0)
    return out.reshape(B, T, 1)
